# revision 15
# baseline (speedup 1.0000x reference)
"""Trainium2 Bass kernel for nn_Attention_40716289966507.

Reference computation (B=4, C=256, H=W=48, heads=8, d=32, N=H*W=2304):
    qkv = w_qkv @ x            # 1x1 conv -> q,k,v each [B, 256, N]
    attn = softmax(q^T k / sqrt(d))   per (batch, head): [N, N]
    out  = v @ attn^T          # [B, 256, N]
    y    = w_proj @ out + b    # [B, 256, N]

Sharding (8 cores): core i handles batch b = i//2 and query-token half
t = i%2 (1152 of the 2304 tokens). Each core needs the full image of its
batch (for K and V) but only its token half for Q; it produces the full
256-channel output for its 1152 tokens, so the host just concatenates.

The kernel is exp-throughput bound (21.2M softmax exponentials per core;
ScalarE does ~153G exp/s). The big lever vs a scalar-only design: a
custom 8-stage DVE op EXP32_ANT computes exp in ONE VectorE instruction,
    exp(z) ~= ((w + A)*w + B)^32,   w = S*z
(quadratic in w then 5 chained squarings; S folded into the host-side q
weights, so PSUM logits ARE w). ScalarE keeps exp(z) = Exp(w * 1/S) via
its free affine. The two engines split the 180 exp tiles ~56/44, roughly
doubling exp throughput; max approx rel err ~5e-3 for |z|<=4.5sigma and
the softmax normalization cancels most of it (validated end-to-end at
rel_err ~1.3e-3).

Per-core pipeline (qkv/proj tiled 3x384 to keep f32r matmuls full-rate;
attention queries tiled 512/512 plus a merged 128-wide tail pass for
both groups -- fp16 S^T is full-rate at any width):
  * qkv in f32r; q/k evacuated to fp16, v transposed [tokens, ch] fp16.
  * per (group of 4 heads, query tile, 128-key chunk):
      - S^T via 4 row-packed fp16 matmuls (K=32 at PE row groups 32h)
        into one PSUM tile pair; logits are pre-scaled to w.
      - exp on ScalarE (scale=1/S) OR VectorE (EXP32_ANT), chosen per
        tile by a deterministic ratio schedule -> pt [128, 4, qt] fp16.
      - AV: 4 col-packed fp16 matmuls accumulating over key chunks
        (f32r cannot col-tile: tile_position=(0,32j) + f32r fails walrus
        codegen, so pt/vT/ones stay fp16).
      - denominators: 4 col-packed ones-matmuls (key-sums replicated
        over each head's 32-partition strip).
      - normalize: RECIPROCAL_APPROX_FAST + one multiply on VectorE.
  * proj: 2 accumulating f32r matmuls; ScalarE adds the per-channel
    bias (Identity activation) while evacuating, DMA out.
PSUM budget is exactly 8 banks: st 2x2 + av 1 + sm 1 + qkv 2.
emit(tc, reps=N) repeats the whole body for the timing NEFF used by
test.py's contention-robust paired estimator (kernel() uses reps=1).
"""

import numpy as np

import concourse.bacc as bacc
import concourse.mybir as mybir
import concourse.tile as tile
from concourse import dve_ops
from concourse.dve_spec import Spec, Src0, C0, C1, sq, lower
from concourse.dve_uop import DveOpSpec

F32 = mybir.dt.float32
F32R = mybir.dt.float32r
FP16 = mybir.dt.float16

P = 128
C = 256          # channels
N = 2304         # tokens per image
NQ = 1152        # query tokens per core
D = 32           # head dim
KC = N // P      # 18 key chunks
QT = 384         # query tile (3 per group); >=256 keeps f32r full-rate
NT = NQ // QT    # 3
SCALE = D ** -0.5

# exp(z) ~= ((S*z + A)*(S*z) + B)^32, minimax-fitted on z in [-6.2, 6.2]
S_EXP = 0.02203952907337898
A_EXP = 1.4221366019241177
B_EXP = 1.0000287417426525

# fraction of exp tiles on ScalarE (rest on VectorE custom op)
ACT_FRAC = 0.56


def _register_exp32():
    name = "EXP32_ANT"
    for op in dve_ops.OPS:
        if op.name == name:
            return op

    def _ref(in0, in1, c0, c1, c2):
        q = ((in0.astype(np.float32) + np.float32(c0)) * in0
             + np.float32(c1)).astype(np.float32)
        for _ in range(5):
            q = (q * q).astype(np.float32)
        return q

    spec = Spec(body=sq(sq(sq(sq(sq((Src0 + C0) * Src0 + C1))))),
                reference=_ref)
    row = max(dve_ops._SUB_OPCODE_FOR_NAME.values()) + 1
    dve_ops._SUB_OPCODE_FOR_NAME[name] = row
    shas = {}
    for ver in ("v3", "v4"):
        shas[ver] = DveOpSpec(name=name, opcode=row,
                              uops=lower(spec, ver=ver),
                              rd1_en=False).sha(ver)
    op = dve_ops.DveOp(name, spec, subdim=False, uops_sha=shas)
    dve_ops.OPS.append(op)
    dve_ops.CUSTOM_DVE_SPECS[name] = spec
    return op


EXP32 = _register_exp32()


def emit(tc, reps=1):
    from contextlib import ExitStack
    ctx = ExitStack()
    nc = tc.nc
    xq_d = nc.dram_tensor("xq", [C, NQ], F32R, kind="ExternalInput").ap()
    xf_d = nc.dram_tensor("xf", [C, N], F32R, kind="ExternalInput").ap()
    wqkvT_d = nc.dram_tensor("wqkvT", [C, 3 * C], F32R, kind="ExternalInput").ap()
    wprojT_d = nc.dram_tensor("wprojT", [C, C], F32R, kind="ExternalInput").ap()
    bprojT_d = nc.dram_tensor("bprojT", [P, 2], F32, kind="ExternalInput").ap()
    y_d = nc.dram_tensor("y", [C, NQ], F32, kind="ExternalOutput").ap()

    singles = ctx.enter_context(tc.tile_pool(name="singles", bufs=1))
    acts = ctx.enter_context(tc.tile_pool(name="acts", bufs=1))
    qkv_ps = ctx.enter_context(tc.tile_pool(name="qkv_ps", bufs=2, space="PSUM"))
    st_ps = ctx.enter_context(tc.tile_pool(name="st_ps", bufs=2, space="PSUM"))
    av_ps = ctx.enter_context(tc.tile_pool(name="av_ps", bufs=1, space="PSUM"))
    sm_ps = ctx.enter_context(tc.tile_pool(name="sm_ps", bufs=1, space="PSUM"))
    pt_pool = ctx.enter_context(tc.tile_pool(name="pt", bufs=4))
    small = ctx.enter_context(tc.tile_pool(name="small", bufs=2))

    # preload the exp table while DMAs/qkv run
    warm = singles.tile([P, 8], F32)
    nc.vector.memset(warm[:], 0.0)
    warm2 = singles.tile([P, 8], F32)
    nc.scalar.activation(warm2[:], warm[:], mybir.ActivationFunctionType.Exp)

    ones_sb = singles.tile([P, D], FP16)
    nc.vector.memset(ones_sb[:], 1.0)
    bias_sb = singles.tile([P, 2], F32)
    nc.sync.dma_start(bias_sb[:], bprojT_d)

    # weights: per-ki-chunk DMAs for early starts
    wq_sb = singles.tile([P, 2, 3 * C], F32R)
    wqkvT_r = wqkvT_d.rearrange("(ki p) o -> p ki o", p=P)
    for sec in range(3):          # q, k, v weight sections separately so
        for ki in range(2):       # the q matmuls start after ~1/3 the bytes
            sl = slice(sec * C, (sec + 1) * C)
            nc.sync.dma_start(wq_sb[:, ki, sl], wqkvT_r[:, ki, sl])
    wp_sb = singles.tile([P, 2, C], F32R)
    nc.sync.dma_start(wp_sb[:], wprojT_d.rearrange("(ki p) o -> p ki o", p=P))

    # x: query half and full image, split by (ki, token range)
    xq_sb = singles.tile([P, 2, NQ], F32R)
    xq_r = xq_d.rearrange("(ki p) n -> p ki n", p=P)
    xf_sb = singles.tile([P, 2, N], F32R)
    xf_r = xf_d.rearrange("(ki p) n -> p ki n", p=P)

    def emit_x_dma():
        for ki in range(2):
            for nt in range(NT):
                sl = slice(nt * QT, (nt + 1) * QT)
                nc.sync.dma_start(xq_sb[:, ki, sl], xq_r[:, ki, sl])
        for ki in range(2):
            for nt in range(N // QT):
                sl = slice(nt * QT, (nt + 1) * QT)
                nc.sync.dma_start(xf_sb[:, ki, sl], xf_r[:, ki, sl])

    # per-group activations
    q_g = [acts.tile([P, NQ], FP16, name=f"q{g}") for g in range(2)]
    k_g = [acts.tile([P, N], FP16, name=f"k{g}") for g in range(2)]
    vT_c = [acts.tile([P, C], FP16, name=f"vt{mo}") for mo in range(KC)]
    av_sb = acts.tile([P, 2, NQ], F32R)
    y_sb = acts.tile([P, 2, NQ], F32)

    mm = nc.tensor.matmul

    # exp engine schedule: deterministic Bresenham on ACT_FRAC
    exp_acc = [0.0]

    def exp_engine():
        exp_acc[0] += ACT_FRAC
        if exp_acc[0] >= 1.0:
            exp_acc[0] -= 1.0
            return "act"
        return "dve"

    def qkv_mm(dst_tile, w_col0, rhs_sb, nt, evac):
        sl = slice(nt * QT, (nt + 1) * QT)
        pst = qkv_ps.tile([P, 512], F32, tag="qkv", name="qkvp")
        ps = pst[:, :QT]
        for ki in range(2):
            mm(ps, wq_sb[:, ki, w_col0:w_col0 + P], rhs_sb[:, ki, sl],
               start=(ki == 0), stop=(ki == 1))
        if evac == "act":
            nc.scalar.copy(dst_tile[:, sl], ps)
        else:
            nc.vector.tensor_copy(dst_tile[:, sl], ps)

    def emit_qkv_group(g):
        # q rows for group g = channels 128g..128g+127; k = 256+128g..
        for nt in range(NT):
            qkv_mm(q_g[g], g * P, xq_sb, nt, "act")
        for nt in range(N // QT):
            qkv_mm(k_g[g], C + g * P, xf_sb, nt, "act")

    def emit_vt(mo):
        pst = qkv_ps.tile([P, 512], F32, tag="qkv", name="qkvp")
        ps = pst[:, :]
        for ki in range(2):
            mm(ps[:, :C], xf_sb[:, ki, mo * P:(mo + 1) * P],
               wq_sb[:, ki, 2 * C:3 * C],
               start=(ki == 0), stop=(ki == 1))
        nc.vector.tensor_copy(vT_c[mo][:], ps[:, :C])

    def emit_attention(g, q0, qtw):
        av = av_ps.tile([P, 512], F32)
        sm = sm_ps.tile([P, 512], F32)
        for kc in range(KC):
            pt = pt_pool.tile([P, 4, 512], FP16)
            for pair in range(2):
                st = st_ps.tile([P, 2, 512], F32, tag="st")
                for hh in range(2):
                    h = 2 * pair + hh
                    mm(st[:, hh, :qtw],
                       k_g[g][32 * h:32 * (h + 1), kc * P:(kc + 1) * P],
                       q_g[g][32 * h:32 * (h + 1), q0:q0 + qtw],
                       start=True, stop=True,
                       tile_position=(32 * h, 0))
                if exp_engine() == "act":
                    nc.scalar.activation(pt[:, 2 * pair:2 * pair + 2, :qtw],
                                         st[:, :, :qtw],
                                         mybir.ActivationFunctionType.Exp,
                                         scale=1.0 / S_EXP)
                else:
                    nc.vector._custom_dve(EXP32,
                                          out=pt[:, 2 * pair:2 * pair + 2, :qtw],
                                          in0=st[:, :, :qtw],
                                          s0=A_EXP, s1=B_EXP)
            for h in range(4):
                mm(av[32 * h:32 * (h + 1), :qtw],
                   vT_c[kc][:, 128 * g + 32 * h:128 * g + 32 * (h + 1)],
                   pt[:, h, :qtw],
                   start=(kc == 0), stop=(kc == KC - 1),
                   tile_position=(0, 32 * h), skip_group_check=True)
            for h in range(4):
                mm(sm[32 * h:32 * (h + 1), :qtw],
                   ones_sb[:, :],
                   pt[:, h, :qtw],
                   start=(kc == 0), stop=(kc == KC - 1),
                   tile_position=(0, 32 * h), skip_group_check=True)
        rec = small.tile([P, 512], F32, tag="rec")
        nc.vector.reciprocal_approx_fast(rec[:, :qtw], sm[:, :qtw])
        nc.vector.tensor_mul(av_sb[:, g, q0:q0 + qtw], av[:, :qtw],
                             rec[:, :qtw])

    def emit_tail():
        # queries 1024:1152 for BOTH groups in one pass: head slot h holds
        # g0 at cols 0:128, g1 at cols 128:256 (fp16 S^T is full-rate at
        # 128-wide, unlike f32r)
        q0, qtw = 1024, 128
        av = av_ps.tile([P, 512], F32)
        sm = sm_ps.tile([P, 512], F32)
        for kc in range(KC):
            pt = pt_pool.tile([P, 4, 512], FP16)
            for pair in range(2):
                st = st_ps.tile([P, 2, 512], F32, tag="st")
                for g in range(2):
                    for hh in range(2):
                        h = 2 * pair + hh
                        mm(st[:, hh, g * qtw:(g + 1) * qtw],
                           k_g[g][32 * h:32 * (h + 1), kc * P:(kc + 1) * P],
                           q_g[g][32 * h:32 * (h + 1), q0:q0 + qtw],
                           start=(g == 0), stop=(g == 1),
                           tile_position=(32 * h, 0), skip_group_check=True)
                if exp_engine() == "act":
                    nc.scalar.activation(pt[:, 2 * pair:2 * pair + 2, :2 * qtw],
                                         st[:, :, :2 * qtw],
                                         mybir.ActivationFunctionType.Exp,
                                         scale=1.0 / S_EXP)
                else:
                    nc.vector._custom_dve(EXP32,
                                          out=pt[:, 2 * pair:2 * pair + 2, :2 * qtw],
                                          in0=st[:, :, :2 * qtw],
                                          s0=A_EXP, s1=B_EXP)
            for g in range(2):
                for h in range(4):
                    mm(av[32 * h:32 * (h + 1), g * qtw:(g + 1) * qtw],
                       vT_c[kc][:, 128 * g + 32 * h:128 * g + 32 * (h + 1)],
                       pt[:, h, g * qtw:(g + 1) * qtw],
                       start=(kc == 0 and g == 0),
                       stop=(kc == KC - 1 and g == 1),
                       tile_position=(0, 32 * h), skip_group_check=True)
            for g in range(2):
                for h in range(4):
                    mm(sm[32 * h:32 * (h + 1), g * qtw:(g + 1) * qtw],
                       ones_sb[:, :],
                       pt[:, h, g * qtw:(g + 1) * qtw],
                       start=(kc == 0 and g == 0),
                       stop=(kc == KC - 1 and g == 1),
                       tile_position=(0, 32 * h), skip_group_check=True)
        rec = small.tile([P, 512], F32, tag="rec")
        nc.vector.reciprocal_approx_fast(rec[:, :2 * qtw], sm[:, :2 * qtw])
        for g in range(2):
            nc.vector.tensor_mul(av_sb[:, g, q0:q0 + qtw],
                                 av[:, g * qtw:(g + 1) * qtw],
                                 rec[:, g * qtw:(g + 1) * qtw])

    y_r = y_d.rearrange("(co p) n -> p co n", p=P)

    def emit_proj(co, nt):
        sl = slice(nt * QT, (nt + 1) * QT)
        pst = qkv_ps.tile([P, 512], F32, tag="qkv", name="qkvp")
        ps = pst[:, :QT]
        for ki in range(2):
            mm(ps, wp_sb[:, ki, co * P:(co + 1) * P],
               av_sb[:, ki, sl],
               start=(ki == 0), stop=(ki == 1))
        nc.scalar.add(y_sb[:, co, sl], ps, bias_sb[:, co:co + 1])
        nc.sync.dma_start(y_r[:, co, sl], y_sb[:, co, sl])

    for _rep in range(reps):
        emit_x_dma()
        emit_qkv_group(0)
        for mo in range(KC):
            emit_vt(mo)
        emit_attention(0, 0, 512)
        emit_qkv_group(1)
        emit_attention(1, 0, 512)
        emit_proj(0, 0)
        emit_proj(1, 0)
        emit_attention(0, 512, 512)
        emit_attention(1, 512, 512)
        emit_proj(0, 1)
        emit_proj(1, 1)
        emit_tail()
        emit_proj(0, 2)
        emit_proj(1, 2)
    ctx.close()


_NC_CACHE = {}


def build_nc(reps=1):
    if reps not in _NC_CACHE:
        nc = bacc.Bacc("TRN2", target_bir_lowering=False, debug=False,
                       num_devices=8)
        with tile.TileContext(nc) as tc:
            emit(tc, reps=reps)
        nc.compile()
        _NC_CACHE[reps] = nc
    return _NC_CACHE[reps]


def build_timing_nc(reps=4):
    return build_nc(reps)


def make_in_maps(x, w_qkv, w_proj, b_proj):
    x = np.ascontiguousarray(np.asarray(x, np.float32)).reshape(4, C, N)
    wqkvT = np.asarray(w_qkv, np.float32).T.copy()
    wqkvT[:, :C] *= np.float32(SCALE * S_EXP)   # fold softmax scale + S into q
    wprojT = np.ascontiguousarray(np.asarray(w_proj, np.float32).T)
    bprojT = np.ascontiguousarray(np.asarray(b_proj, np.float32).reshape(2, P).T)
    in_maps = []
    for core in range(8):
        b, t = divmod(core, 2)
        in_maps.append({
            "xq": np.ascontiguousarray(x[b][:, t * NQ:(t + 1) * NQ]),
            "xf": x[b],
            "wqkvT": wqkvT,
            "wprojT": wprojT,
            "bprojT": bprojT,
        })
    return in_maps


def assemble_output(results):
    y = np.empty((4, C, N), np.float32)
    for core in range(8):
        b, t = divmod(core, 2)
        y[b][:, t * NQ:(t + 1) * NQ] = results[core]["y"]
    return y.reshape(4, C, 48, 48)


def kernel(x, w_qkv, w_proj, b_proj):
    from concourse.bass_utils import run_bass_kernel_spmd
    nc = build_nc()
    in_maps = make_in_maps(x, w_qkv, w_proj, b_proj)
    res = run_bass_kernel_spmd(nc, in_maps, core_ids=list(range(8)))
    return assemble_output(res.results)


# revision 16
# speedup vs baseline: 1.1652x; 1.1652x over previous
"""Trainium2 Bass kernel for nn_Attention_40716289966507.

Reference computation (B=4, C=256, H=W=48, heads=8, d=32, N=H*W=2304):
    qkv = w_qkv @ x            # 1x1 conv -> q,k,v each [B, 256, N]
    attn = softmax(q^T k / sqrt(d))   per (batch, head): [N, N]
    out  = v @ attn^T          # [B, 256, N]
    y    = w_proj @ out + b    # [B, 256, N]

Sharding (8 cores): core i handles batch b = i//2 and query-token half
t = i%2 (1152 of the 2304 tokens). Each core needs the full image of its
batch (for K and V) but only its token half for Q; it produces the full
256-channel output for its 1152 tokens, so the host just concatenates.

The kernel is exp-throughput bound (21.2M softmax exponentials per core;
ScalarE does ~153G exp/s). The big lever vs a scalar-only design: a
custom 8-stage DVE op EXP32_ANT computes exp in ONE VectorE instruction,
    exp(z) ~= ((w + A)*w + B)^32,   w = S*z
(quadratic in w then 5 chained squarings; S folded into the host-side q
weights, so PSUM logits ARE w). ScalarE keeps exp(z) = Exp(w * 1/S) via
its free affine. The two engines split the 180 exp tiles ~56/44, roughly
doubling exp throughput; max approx rel err ~5e-3 for |z|<=4.5sigma and
the softmax normalization cancels most of it (validated end-to-end at
rel_err ~1.3e-3).

Per-core pipeline (qkv/proj tiled 3x384 to keep f32r matmuls full-rate;
attention queries tiled 512/512 plus a merged 128-wide tail pass for
both groups -- fp16 S^T is full-rate at any width):
  * qkv in f32r; q/k evacuated to fp16, v transposed [tokens, ch] fp16.
  * per (group of 4 heads, query tile, 128-key chunk):
      - S^T via 4 row-packed fp16 matmuls (K=32 at PE row groups 32h)
        into one PSUM tile pair; logits are pre-scaled to w.
      - exp on ScalarE (scale=1/S) OR VectorE (EXP32_ANT), chosen per
        tile by a deterministic ratio schedule -> pt [128, 4, qt] fp16.
      - AV: 4 col-packed fp16 matmuls accumulating over key chunks
        (f32r cannot col-tile: tile_position=(0,32j) + f32r fails walrus
        codegen, so pt/vT/ones stay fp16).
      - denominators: 4 col-packed ones-matmuls (key-sums replicated
        over each head's 32-partition strip).
      - normalize: RECIPROCAL_APPROX_FAST + one multiply on VectorE.
  * proj: 2 accumulating f32r matmuls; ScalarE adds the per-channel
    bias (Identity activation) while evacuating, DMA out.
PSUM budget is exactly 8 banks: st 2x2 + av 1 + sm 1 + qkv 2.
emit(tc, reps=N) repeats the whole body for the timing NEFF used by
test.py's contention-robust paired estimator (kernel() uses reps=1).
"""

import numpy as np

import concourse.bacc as bacc
import concourse.mybir as mybir
import concourse.tile as tile
from concourse import dve_ops
from concourse.dve_spec import Spec, Src0, C0, C1, sq, lower
from concourse.dve_uop import DveOpSpec

F32 = mybir.dt.float32
F32R = mybir.dt.float32r
FP16 = mybir.dt.float16

P = 128
C = 256          # channels
N = 2304         # tokens per image
NQ = 1152        # query tokens per core
D = 32           # head dim
KC = N // P      # 18 key chunks
QT = 384         # query tile (3 per group); >=256 keeps f32r full-rate
NT = NQ // QT    # 3
SCALE = D ** -0.5

# exp(z) ~= ((S*z + A)*(S*z) + B)^32, minimax-fitted on z in [-6.2, 6.2]
S_EXP = 0.02203952907337898
A_EXP = 1.4221366019241177
B_EXP = 1.0000287417426525

# fraction of exp tiles on ScalarE (rest on VectorE custom op)
ACT_FRAC = 0.54


def _register_exp32():
    name = "EXP32_ANT"
    for op in dve_ops.OPS:
        if op.name == name:
            return op

    def _ref(in0, in1, c0, c1, c2):
        q = ((in0.astype(np.float32) + np.float32(c0)) * in0
             + np.float32(c1)).astype(np.float32)
        for _ in range(5):
            q = (q * q).astype(np.float32)
        return q

    spec = Spec(body=sq(sq(sq(sq(sq((Src0 + C0) * Src0 + C1))))),
                reference=_ref)
    row = max(dve_ops._SUB_OPCODE_FOR_NAME.values()) + 1
    dve_ops._SUB_OPCODE_FOR_NAME[name] = row
    shas = {}
    for ver in ("v3", "v4"):
        shas[ver] = DveOpSpec(name=name, opcode=row,
                              uops=lower(spec, ver=ver),
                              rd1_en=False).sha(ver)
    op = dve_ops.DveOp(name, spec, subdim=False, uops_sha=shas)
    dve_ops.OPS.append(op)
    dve_ops.CUSTOM_DVE_SPECS[name] = spec
    return op


EXP32 = _register_exp32()


def emit(tc, reps=1):
    from contextlib import ExitStack
    ctx = ExitStack()
    nc = tc.nc
    xq_d = nc.dram_tensor("xq", [C, NQ], F32R, kind="ExternalInput").ap()
    xf_d = nc.dram_tensor("xf", [C, N], F32R, kind="ExternalInput").ap()
    wqkvT_d = nc.dram_tensor("wqkvT", [C, 3 * C], F32R, kind="ExternalInput").ap()
    wprojT_d = nc.dram_tensor("wprojT", [C, C], F32R, kind="ExternalInput").ap()
    bprojT_d = nc.dram_tensor("bprojT", [P, 2], F32, kind="ExternalInput").ap()
    y_d = nc.dram_tensor("y", [C, NQ], F32, kind="ExternalOutput").ap()

    singles = ctx.enter_context(tc.tile_pool(name="singles", bufs=1))
    acts = ctx.enter_context(tc.tile_pool(name="acts", bufs=1))
    qkv_ps = ctx.enter_context(tc.tile_pool(name="qkv_ps", bufs=2, space="PSUM"))
    st_ps = ctx.enter_context(tc.tile_pool(name="st_ps", bufs=2, space="PSUM"))
    av_ps = ctx.enter_context(tc.tile_pool(name="av_ps", bufs=1, space="PSUM"))
    sm_ps = ctx.enter_context(tc.tile_pool(name="sm_ps", bufs=1, space="PSUM"))
    pt_pool = ctx.enter_context(tc.tile_pool(name="pt", bufs=4))
    small = ctx.enter_context(tc.tile_pool(name="small", bufs=2))

    # preload the exp table while DMAs/qkv run
    warm = singles.tile([P, 8], F32)
    nc.vector.memset(warm[:], 0.0)
    warm2 = singles.tile([P, 8], F32)
    nc.scalar.activation(warm2[:], warm[:], mybir.ActivationFunctionType.Exp)

    ones_sb = singles.tile([P, D], FP16)
    nc.vector.memset(ones_sb[:], 1.0)
    bias_sb = singles.tile([P, 2], F32)
    nc.sync.dma_start(bias_sb[:], bprojT_d)

    # weights: per-ki-chunk DMAs for early starts
    wq_sb = singles.tile([P, 2, 3 * C], F32R)
    wqkvT_r = wqkvT_d.rearrange("(ki p) o -> p ki o", p=P)
    for sec in range(3):          # q, k, v weight sections separately so
        for ki in range(2):       # the q matmuls start after ~1/3 the bytes
            sl = slice(sec * C, (sec + 1) * C)
            nc.sync.dma_start(wq_sb[:, ki, sl], wqkvT_r[:, ki, sl])
    wp_sb = singles.tile([P, 2, C], F32R)
    nc.sync.dma_start(wp_sb[:], wprojT_d.rearrange("(ki p) o -> p ki o", p=P))

    # x: query half and full image, split by (ki, token range)
    xq_sb = singles.tile([P, 2, NQ], F32R)
    xq_r = xq_d.rearrange("(ki p) n -> p ki n", p=P)
    xf_sb = singles.tile([P, 2, N], F32R)
    xf_r = xf_d.rearrange("(ki p) n -> p ki n", p=P)

    def emit_x_dma():
        for ki in range(2):
            for nt in range(NT):
                sl = slice(nt * QT, (nt + 1) * QT)
                nc.sync.dma_start(xq_sb[:, ki, sl], xq_r[:, ki, sl])
        for ki in range(2):
            for nt in range(N // QT):
                sl = slice(nt * QT, (nt + 1) * QT)
                nc.sync.dma_start(xf_sb[:, ki, sl], xf_r[:, ki, sl])

    # per-group activations
    q_g = [acts.tile([P, NQ], FP16, name=f"q{g}") for g in range(2)]
    k_g = [acts.tile([P, N], FP16, name=f"k{g}") for g in range(2)]
    vT_c = [acts.tile([P, C], FP16, name=f"vt{mo}") for mo in range(KC)]
    av_sb = acts.tile([P, 2, NQ], F32R)
    y_sb = acts.tile([P, 2, NQ], F32)

    mm = nc.tensor.matmul

    # exp engine schedule: deterministic Bresenham on ACT_FRAC
    exp_acc = [0.0]

    def exp_engine():
        exp_acc[0] += ACT_FRAC
        if exp_acc[0] >= 1.0:
            exp_acc[0] -= 1.0
            return "act"
        return "dve"

    def qkv_mm(dst_tile, w_col0, rhs_sb, nt, evac):
        sl = slice(nt * QT, (nt + 1) * QT)
        pst = qkv_ps.tile([P, 512], F32, tag="qkv", name="qkvp")
        ps = pst[:, :QT]
        for ki in range(2):
            mm(ps, wq_sb[:, ki, w_col0:w_col0 + P], rhs_sb[:, ki, sl],
               start=(ki == 0), stop=(ki == 1))
        if evac == "act":
            nc.scalar.copy(dst_tile[:, sl], ps)
        else:
            nc.vector.tensor_copy(dst_tile[:, sl], ps)

    def emit_qkv_group(g):
        # q rows for group g = channels 128g..128g+127; k = 256+128g..
        for nt in range(NT):
            qkv_mm(q_g[g], g * P, xq_sb, nt, "dve")
        for nt in range(N // QT):
            qkv_mm(k_g[g], C + g * P, xf_sb, nt, "act")

    def emit_vt(mo):
        pst = qkv_ps.tile([P, 512], F32, tag="qkv", name="qkvp")
        ps = pst[:, :]
        for ki in range(2):
            mm(ps[:, :C], xf_sb[:, ki, mo * P:(mo + 1) * P],
               wq_sb[:, ki, 2 * C:3 * C],
               start=(ki == 0), stop=(ki == 1))
        nc.vector.tensor_copy(vT_c[mo][:], ps[:, :C])

    def emit_attention(g, q0, qtw):
        av = av_ps.tile([P, 512], F32)
        sm = sm_ps.tile([P, 512], F32)
        for kc in range(KC):
            pt = pt_pool.tile([P, 4, 512], FP16)
            for pair in range(2):
                st = st_ps.tile([P, 2, 512], F32, tag="st")
                for hh in range(2):
                    h = 2 * pair + hh
                    mm(st[:, hh, :qtw],
                       k_g[g][32 * h:32 * (h + 1), kc * P:(kc + 1) * P],
                       q_g[g][32 * h:32 * (h + 1), q0:q0 + qtw],
                       start=True, stop=True,
                       tile_position=(32 * h, 0))
                if exp_engine() == "act":
                    nc.scalar.activation(pt[:, 2 * pair:2 * pair + 2, :qtw],
                                         st[:, :, :qtw],
                                         mybir.ActivationFunctionType.Exp,
                                         scale=1.0 / S_EXP)
                else:
                    nc.vector._custom_dve(EXP32,
                                          out=pt[:, 2 * pair:2 * pair + 2, :qtw],
                                          in0=st[:, :, :qtw],
                                          s0=A_EXP, s1=B_EXP)
            for h in range(4):
                mm(av[32 * h:32 * (h + 1), :qtw],
                   vT_c[kc][:, 128 * g + 32 * h:128 * g + 32 * (h + 1)],
                   pt[:, h, :qtw],
                   start=(kc == 0), stop=(kc == KC - 1),
                   tile_position=(0, 32 * h), skip_group_check=True)
            for h in range(4):
                mm(sm[32 * h:32 * (h + 1), :qtw],
                   ones_sb[:, :],
                   pt[:, h, :qtw],
                   start=(kc == 0), stop=(kc == KC - 1),
                   tile_position=(0, 32 * h), skip_group_check=True)
        rec = small.tile([P, 512], F32, tag="rec")
        nc.vector.reciprocal_approx_fast(rec[:, :qtw], sm[:, :qtw])
        nc.vector.tensor_mul(av_sb[:, g, q0:q0 + qtw], av[:, :qtw],
                             rec[:, :qtw])

    def emit_tail():
        # queries 1024:1152 for BOTH groups in one pass: head slot h holds
        # g0 at cols 0:128, g1 at cols 128:256 (fp16 S^T is full-rate at
        # 128-wide, unlike f32r)
        q0, qtw = 1024, 128
        av = av_ps.tile([P, 512], F32)
        sm = sm_ps.tile([P, 512], F32)
        for kc in range(KC):
            pt = pt_pool.tile([P, 4, 512], FP16)
            for pair in range(2):
                st = st_ps.tile([P, 2, 512], F32, tag="st")
                for g in range(2):
                    for hh in range(2):
                        h = 2 * pair + hh
                        mm(st[:, hh, g * qtw:(g + 1) * qtw],
                           k_g[g][32 * h:32 * (h + 1), kc * P:(kc + 1) * P],
                           q_g[g][32 * h:32 * (h + 1), q0:q0 + qtw],
                           start=(g == 0), stop=(g == 1),
                           tile_position=(32 * h, 0), skip_group_check=True)
                if exp_engine() == "act":
                    nc.scalar.activation(pt[:, 2 * pair:2 * pair + 2, :2 * qtw],
                                         st[:, :, :2 * qtw],
                                         mybir.ActivationFunctionType.Exp,
                                         scale=1.0 / S_EXP)
                else:
                    nc.vector._custom_dve(EXP32,
                                          out=pt[:, 2 * pair:2 * pair + 2, :2 * qtw],
                                          in0=st[:, :, :2 * qtw],
                                          s0=A_EXP, s1=B_EXP)
            for g in range(2):
                for h in range(4):
                    mm(av[32 * h:32 * (h + 1), g * qtw:(g + 1) * qtw],
                       vT_c[kc][:, 128 * g + 32 * h:128 * g + 32 * (h + 1)],
                       pt[:, h, g * qtw:(g + 1) * qtw],
                       start=(kc == 0 and g == 0),
                       stop=(kc == KC - 1 and g == 1),
                       tile_position=(0, 32 * h), skip_group_check=True)
            for g in range(2):
                for h in range(4):
                    mm(sm[32 * h:32 * (h + 1), g * qtw:(g + 1) * qtw],
                       ones_sb[:, :],
                       pt[:, h, g * qtw:(g + 1) * qtw],
                       start=(kc == 0 and g == 0),
                       stop=(kc == KC - 1 and g == 1),
                       tile_position=(0, 32 * h), skip_group_check=True)
        rec = small.tile([P, 512], F32, tag="rec")
        nc.vector.reciprocal_approx_fast(rec[:, :2 * qtw], sm[:, :2 * qtw])
        for g in range(2):
            nc.vector.tensor_mul(av_sb[:, g, q0:q0 + qtw],
                                 av[:, g * qtw:(g + 1) * qtw],
                                 rec[:, g * qtw:(g + 1) * qtw])

    y_r = y_d.rearrange("(co p) n -> p co n", p=P)

    def emit_proj(co, nt):
        sl = slice(nt * QT, (nt + 1) * QT)
        pst = qkv_ps.tile([P, 512], F32, tag="qkv", name="qkvp")
        ps = pst[:, :QT]
        for ki in range(2):
            mm(ps, wp_sb[:, ki, co * P:(co + 1) * P],
               av_sb[:, ki, sl],
               start=(ki == 0), stop=(ki == 1))
        nc.scalar.add(y_sb[:, co, sl], ps, bias_sb[:, co:co + 1])
        nc.sync.dma_start(y_r[:, co, sl], y_sb[:, co, sl])

    for _rep in range(reps):
        emit_x_dma()
        emit_qkv_group(0)
        for mo in range(KC):
            emit_vt(mo)
        emit_attention(0, 0, 512)
        emit_qkv_group(1)
        emit_attention(1, 0, 512)
        emit_proj(0, 0)
        emit_proj(1, 0)
        emit_attention(0, 512, 512)
        emit_attention(1, 512, 512)
        emit_proj(0, 1)
        emit_proj(1, 1)
        emit_tail()
        emit_proj(0, 2)
        emit_proj(1, 2)
    ctx.close()


_NC_CACHE = {}


def build_nc(reps=1):
    if reps not in _NC_CACHE:
        nc = bacc.Bacc("TRN2", target_bir_lowering=False, debug=False,
                       num_devices=8)
        with tile.TileContext(nc) as tc:
            emit(tc, reps=reps)
        nc.compile()
        _NC_CACHE[reps] = nc
    return _NC_CACHE[reps]


def build_timing_nc(reps=4):
    return build_nc(reps)


def make_in_maps(x, w_qkv, w_proj, b_proj):
    x = np.ascontiguousarray(np.asarray(x, np.float32)).reshape(4, C, N)
    wqkvT = np.asarray(w_qkv, np.float32).T.copy()
    wqkvT[:, :C] *= np.float32(SCALE * S_EXP)   # fold softmax scale + S into q
    wprojT = np.ascontiguousarray(np.asarray(w_proj, np.float32).T)
    bprojT = np.ascontiguousarray(np.asarray(b_proj, np.float32).reshape(2, P).T)
    in_maps = []
    for core in range(8):
        b, t = divmod(core, 2)
        in_maps.append({
            "xq": np.ascontiguousarray(x[b][:, t * NQ:(t + 1) * NQ]),
            "xf": x[b],
            "wqkvT": wqkvT,
            "wprojT": wprojT,
            "bprojT": bprojT,
        })
    return in_maps


def assemble_output(results):
    y = np.empty((4, C, N), np.float32)
    for core in range(8):
        b, t = divmod(core, 2)
        y[b][:, t * NQ:(t + 1) * NQ] = results[core]["y"]
    return y.reshape(4, C, 48, 48)


def kernel(x, w_qkv, w_proj, b_proj):
    from concourse.bass_utils import run_bass_kernel_spmd
    nc = build_nc()
    in_maps = make_in_maps(x, w_qkv, w_proj, b_proj)
    res = run_bass_kernel_spmd(nc, in_maps, core_ids=list(range(8)))
    return assemble_output(res.results)
